# revision 18
# baseline (speedup 1.0000x reference)
"""Trainium2 Bass kernel for sigmoid-projection strictly-causal attention.

Reference computation (B=8, S=2048, D=512, U=512):
    q = sigmoid(x @ Wq); k = sigmoid(x @ Wv); v = sigmoid(x @ Wk)
    score = (q @ k^T) / sqrt(D)                       [S, S]
    mask: strictly causal (key j < query i); row 0 -> zeros
    out = softmax(score) @ v                          [S, U]

Sharding: data-parallel over batch, one batch element per NeuronCore
(8 cores), weights replicated, no collectives.  Full inputs in, full
[B, S, U] output back.

Per-core kernel (all matmuls fp8e4m3 DoubleRow, 2x PE throughput):
  - The host pre-packs X^T and the three weight matrices into fp8 in
    the DR-paired [128, pair, free] layouts, so the kernel starts with
    ~1.75 MiB of plain DMA (split into pieces across the queues so the
    first projection can start after ~0.25 MiB) and the PE never
    transposes anything.
  - Projections: Q^T/K^T [u, s] with the weight pairs stationary
    (reused across all four s-chunks), V [s, u] with X^T pairs
    stationary.  PSUM tiles span 4 banks so one sigmoid evicts 2048
    columns (fewer ACT instructions), writing fp8.
  - Scores are built transposed, S^T[k, q] = K^T-block-stationary @
    Q^T, per key-block j over all later q-chunks (stationary reused
    across chunks).  A strict-causal additive mask covers the diagonal
    block; exp folds 1/sqrt(D) and a -8 bias (keeps fp8 in range; the
    shift cancels in softmax), writing P'^T in fp8, two key-blocks per
    instruction off a 2-bank PSUM pair except on the ragged diagonal.
    Sub-diagonal gaps are zero-filled so PV can run on full pairs.
  - Denominators: ones-stationary DR matmul over P'^T pairs (row 0 of
    a broadcast [128, 512] result), streamed to DRAM per chunk; the
    host divides, so the denominator never needs an on-device
    transpose.
  - PV: P'^T pair slices stationary, V pairs moving -> out[q, u] in
    natural orientation, evicted bf16 and DMA'd per 128-row tile.
"""

import sys

for _p in ("/opt/trn_rl_repo",):
    if _p not in sys.path:
        sys.path.insert(0, _p)

import numpy as np

B, S, D, U = 8, 2048, 512, 512
P = 128
NCORES = 8
DT = D // P   # 4 d-tiles
UT = U // P   # 4 u-tiles
ST = S // P   # 16 s-tiles
SC = S // 512  # 4 s-chunks
C_SHIFT = 8.0  # exp(s - C): keeps P' well inside fp8e4m3 range

_cache = {}


def _build():
    import concourse.mybir as mybir
    import concourse.tile as tile
    from concourse import bacc

    f32 = mybir.dt.float32
    bf16 = mybir.dt.bfloat16
    f8 = mybir.dt.float8e4
    AF = mybir.ActivationFunctionType
    DR = mybir.MatmulPerfMode.DoubleRow

    nc = bacc.Bacc("TRN2", target_bir_lowering=False, debug=False,
                   num_devices=NCORES)

    xt8_ext = nc.dram_tensor("xt8", [P, DT, S], f8, kind="ExternalInput")
    wq8_ext = nc.dram_tensor("wq8", [P, DT, U], f8, kind="ExternalInput")
    wv8_ext = nc.dram_tensor("wv8", [P, DT, U], f8, kind="ExternalInput")
    wk8_ext = nc.dram_tensor("wk8", [P, DT, U], f8, kind="ExternalInput")
    # out stored partition-major [p, s_tile, u] so each partition's DMA
    # run is 16 KiB contiguous (1 KiB rows would throttle the queues to
    # ~21 GB/s); the host unpermutes.
    out_ext = nc.dram_tensor("out", [P, ST, U], bf16, kind="ExternalOutput")
    den_ext = nc.dram_tensor("den", [1, S], f32, kind="ExternalOutput")

    # [k_p, q_f] additive mask for the diagonal block: keep (0) where
    # k < q strictly, -1e30 elsewhere.
    mask_dram = nc.inline_tensor(
        np.where(np.triu(np.ones((P, P), bool), 1), 0.0, -1e30)
        .astype(np.float32), "maskT_const")

    inv_sqrt_d = 1.0 / float(np.sqrt(D))

    with tile.TileContext(nc) as tc:
        with (
            tc.tile_pool(name="const", bufs=1) as constp,
            tc.tile_pool(name="inp", bufs=1) as inp,
            tc.tile_pool(name="proj", bufs=1) as projp,
            tc.tile_pool(name="pt", bufs=1) as ptp,
            tc.tile_pool(name="dhold", bufs=2) as dholdp,
        ):
            maskT = constp.tile([P, P], f32)
            nc.gpsimd.dma_start(maskT[:], mask_dram[:])
            # dual-fp8 LDWEIGHTS requires the standard paired stationary
            # shape; a [P, 2, 1] ones AP fails the ISA check, so keep a
            # [P, 2, 512] ones tile and use an M=128 slice (the denom
            # matmul then just produces 128 identical rows).
            ones8 = constp.tile([P, 2, 512], f8)
            nc.vector.memset(ones8[:], 1.0)
            nbias = constp.tile([P, 1], f32)
            nc.vector.memset(nbias[:], -C_SHIFT)

            # ---- input DMAs: everything partition-split across both
            # HWDGE queues (keeps per-partition runs contiguous -> big
            # DMA packets).  wq8 leads (first projection group), X^T
            # next, the later-needed weights after. ----
            w8 = {n: inp.tile([P, DT, U], f8, name=f"w8_{n}")
                  for n in ("q", "v", "k")}
            xt8 = inp.tile([P, DT, S], f8, name="xt8")

            for lo, hi, qeng in ((0, 64, nc.sync), (64, P, nc.scalar)):
                qeng.dma_start(w8["q"][lo:hi], wq8_ext[lo:hi])
                qeng.dma_start(xt8[lo:hi, 0:2, :], xt8_ext[lo:hi, 0:2, :])
                qeng.dma_start(xt8[lo:hi, 2:4, :], xt8_ext[lo:hi, 2:4, :])
                qeng.dma_start(w8["v"][lo:hi], wv8_ext[lo:hi])
                qeng.dma_start(w8["k"][lo:hi], wk8_ext[lo:hi])

            qT8 = projp.tile([P, UT, S], f8, name="qT8")
            kT8 = projp.tile([P, UT, S], f8, name="kT8")
            v8 = projp.tile([P, ST, U], f8, name="v8")
            outS = projp.tile([P, ST, U], bf16, name="outS")

            # P'^T pair tiles: pair jj covers key blocks (2jj, 2jj+1),
            # per 512-query chunk qc >= jj//2
            pt8 = {}
            for qc in range(SC):
                for jj in range(2 * qc + 2):
                    pt8[(jj, qc)] = ptp.tile([P, 2, 512], f8,
                                             name=f"pt8_{jj}_{qc}")

            denomT = dholdp.tile([1, S], f32, name="denomT")

            # ---- phase 1: projections (4-bank PSUM tiles, one sigmoid
            # per 2048 columns) ----
            with tc.tile_pool(name="bigps", bufs=2, space="PSUM") as bigps:
                for dst, wkey in ((qT8, "q"), (kT8, "v")):
                    for u in range(UT):
                        ps = bigps.tile([P, SC, 512], f32, tag="big",
                                        name=f"ps_{u}_{wkey}")
                        for tt in (0, 2):
                            for c in range(SC):
                                nc.tensor.matmul(
                                    ps[:, c, :],
                                    w8[wkey][:, tt:tt + 2,
                                             u * P:(u + 1) * P],
                                    xt8[:, tt:tt + 2,
                                        c * 512:(c + 1) * 512],
                                    start=(tt == 0), stop=(tt == 2),
                                    perf_mode=DR)
                        nc.scalar.activation(out=dst[:, u, :], in_=ps[:],
                                             func=AF.Sigmoid)

                for sg in range(4):  # V, groups of 4 s-tiles
                    ps = bigps.tile([P, 4, 512], f32, tag="big",
                                    name=f"ps_v_{sg}")
                    for si in range(4):
                        st = 4 * sg + si
                        for tt in (0, 2):
                            nc.tensor.matmul(
                                ps[:, si, :],
                                xt8[:, tt:tt + 2, st * P:(st + 1) * P],
                                w8["k"][:, tt:tt + 2, :],
                                start=(tt == 0), stop=(tt == 2),
                                perf_mode=DR)
                    nc.scalar.activation(out=v8[:, 4 * sg:4 * sg + 4, :],
                                         in_=ps[:], func=AF.Sigmoid)

            # ---- phase 2: attention ----
            with (
                tc.tile_pool(name="sps", bufs=3, space="PSUM") as sps,
                tc.tile_pool(name="pvps", bufs=2, space="PSUM") as pvps,
            ):
                # scores S^T[k,q] grouped by key block j (stationary K^T
                # pair reused across q-chunks); after key-group g
                # finishes, chunk qc=g is fully scored -> denom + PV.
                def emit_scores(g):
                    for jj in (2 * g, 2 * g + 1):
                        ps = sps.tile([P, 2, 512], f32, tag="sps",
                                      name=f"ps_s_{jj}_{g}")
                        for hf in range(2):
                            j = 2 * jj + hf
                            r = j - 4 * g
                            # diagonal chunk (compact at col 0)
                            w = 512 - P * r
                            for uu in (0, 2):
                                nc.tensor.matmul(
                                    ps[:, hf, :w],
                                    kT8[:, uu:uu + 2, j * P:(j + 1) * P],
                                    qT8[:, uu:uu + 2,
                                        g * 512 + P * r:(g + 1) * 512],
                                    start=(uu == 0), stop=(uu == 2),
                                    perf_mode=DR)
                            nc.vector.tensor_add(
                                out=ps[:, hf, 0:P], in0=ps[:, hf, 0:P],
                                in1=maskT[:])
                            nc.scalar.activation(
                                out=pt8[(jj, g)][:, hf, P * r:512],
                                in_=ps[:, hf, :w], func=AF.Exp,
                                scale=inv_sqrt_d, bias=nbias[:, 0:1])
                            if r:
                                nc.vector.memset(
                                    pt8[(jj, g)][:, hf, 0:P * r], 0.0)
                        # later chunks: full width, exp two blocks at once
                        for qc in range(g + 1, SC):
                            ps = sps.tile([P, 2, 512], f32, tag="sps",
                                          name=f"ps_s_{jj}_{qc}")
                            for hf in range(2):
                                j = 2 * jj + hf
                                for uu in (0, 2):
                                    nc.tensor.matmul(
                                        ps[:, hf, :],
                                        kT8[:, uu:uu + 2,
                                            j * P:(j + 1) * P],
                                        qT8[:, uu:uu + 2,
                                            qc * 512:(qc + 1) * 512],
                                        start=(uu == 0), stop=(uu == 2),
                                        perf_mode=DR)
                            nc.scalar.activation(
                                out=pt8[(jj, qc)][:], in_=ps[:],
                                func=AF.Exp, scale=inv_sqrt_d,
                                bias=nbias[:, 0:1])

                def emit_chunk(qc):
                    npairs = 2 * qc + 2
                    # softmax denominators for this chunk
                    dn = pvps.tile([P, 512], f32, tag="pv",
                                   name=f"dn_{qc}")
                    for jj in range(npairs):
                        nc.tensor.matmul(
                            dn[:], ones8[:, :, 0:P], pt8[(jj, qc)][:],
                            start=(jj == 0), stop=(jj == npairs - 1),
                            perf_mode=DR)
                    nc.vector.tensor_copy(
                        out=denomT[:, qc * 512:(qc + 1) * 512],
                        in_=dn[0:1, :])
                    nc.gpsimd.dma_start(
                        den_ext[:, qc * 512:(qc + 1) * 512],
                        denomT[:, qc * 512:(qc + 1) * 512])
                    # PV: P'^T slice stationary -> natural [q, u]
                    for it in range(4):
                        i = 4 * qc + it
                        np_i = (i + 2) // 2
                        po = pvps.tile([P, U], f32, tag="pv")
                        for jj in range(np_i):
                            nc.tensor.matmul(
                                po[:],
                                pt8[(jj, qc)][:, :, it * P:(it + 1) * P],
                                v8[:, 2 * jj:2 * jj + 2, :],
                                start=(jj == 0), stop=(jj == np_i - 1),
                                perf_mode=DR)
                        nc.vector.tensor_copy(out=outS[:, i, :], in_=po[:])
                    lo, hi = 4 * qc, 4 * qc + 4
                    nc.sync.dma_start(out_ext[0:64, lo:hi, :],
                                      outS[0:64, lo:hi, :])
                    nc.scalar.dma_start(out_ext[64:P, lo:hi, :],
                                        outS[64:P, lo:hi, :])

                for g in range(SC):
                    emit_scores(g)
                    emit_chunk(g)

    nc.compile()
    return nc


def _get_nc():
    if "nc" not in _cache:
        _cache["nc"] = _build()
    return _cache["nc"]


def _prep_in_maps(query, Wq, Wv, Wk):
    import ml_dtypes

    f8 = ml_dtypes.float8_e4m3
    # X^T packed [128, 4, 2048]: xt8[p, t, s] = X[s, 128t + p]
    xt = np.ascontiguousarray(
        np.asarray(query, dtype=np.float32).transpose(0, 2, 1)
        .reshape(B, DT, P, S).transpose(0, 2, 1, 3)).astype(f8)
    ws = []
    for w in (Wq, Wv, Wk):
        ws.append(np.ascontiguousarray(
            np.asarray(w, dtype=np.float32)
            .reshape(DT, P, U).transpose(1, 0, 2)).astype(f8))
    wq8, wv8, wk8 = ws
    return [
        {"xt8": xt[b], "wq8": wq8, "wv8": wv8, "wk8": wk8}
        for b in range(B)
    ]


def _postprocess(res):
    out = np.empty((B, S, U), dtype=np.float32)
    for b in range(B):
        o = (np.asarray(res.results[b]["out"]).astype(np.float32)
             .transpose(1, 0, 2).reshape(S, U))
        den = np.asarray(res.results[b]["den"]).astype(np.float32)
        out[b] = o / np.maximum(den.reshape(S, 1), 1e-30)
    return out


def kernel(query, Wq, Wv, Wk):
    from concourse.bass_utils import run_bass_kernel_spmd

    nc = _get_nc()
    in_maps = _prep_in_maps(query, Wq, Wv, Wk)
    res = run_bass_kernel_spmd(nc, in_maps, core_ids=list(range(NCORES)))
    return _postprocess(res)


# revision 24
# speedup vs baseline: 1.1864x; 1.1864x over previous
"""Trainium2 Bass kernel for sigmoid-projection strictly-causal attention.

Reference computation (B=8, S=2048, D=512, U=512):
    q = sigmoid(x @ Wq); k = sigmoid(x @ Wv); v = sigmoid(x @ Wk)
    score = (q @ k^T) / sqrt(D)                       [S, S]
    mask: strictly causal (key j < query i); row 0 -> zeros
    out = softmax(score) @ v                          [S, U]

Sharding: data-parallel over batch, one batch element per NeuronCore
(8 cores), weights replicated, no collectives.  Full inputs in, full
[B, S, U] output back.

Per-core kernel (all matmuls fp8e4m3 DoubleRow, 2x PE throughput):
  - The host pre-packs X^T and the three weight matrices into fp8 in
    the DR-paired [128, pair, free] layouts, so the kernel starts with
    ~1.75 MiB of plain DMA (split into pieces across the queues so the
    first projection can start after ~0.25 MiB) and the PE never
    transposes anything.
  - Projections: Q^T/K^T [u, s] with the weight pairs stationary
    (reused across all four s-chunks), V [s, u] with X^T pairs
    stationary.  PSUM tiles span 4 banks so one sigmoid evicts 2048
    columns (fewer ACT instructions), writing fp8.
  - Scores are built transposed, S^T[k, q] = K^T-block-stationary @
    Q^T, per key-block j over all later q-chunks (stationary reused
    across chunks).  A strict-causal additive mask covers the diagonal
    block; exp folds 1/sqrt(D) and a -8 bias (keeps fp8 in range; the
    shift cancels in softmax), writing P'^T in fp8, two key-blocks per
    instruction off a 2-bank PSUM pair except on the ragged diagonal.
    Sub-diagonal gaps are zero-filled so PV can run on full pairs.
  - Denominators: ones-stationary DR matmul over P'^T pairs (row 0 of
    a broadcast [128, 512] result), streamed to DRAM per chunk; the
    host divides, so the denominator never needs an on-device
    transpose.
  - PV: P'^T pair slices stationary, V pairs moving -> out[q, u] in
    natural orientation, evicted bf16 and DMA'd per 128-row tile.
"""

import sys

for _p in ("/opt/trn_rl_repo",):
    if _p not in sys.path:
        sys.path.insert(0, _p)

import numpy as np

B, S, D, U = 8, 2048, 512, 512
P = 128
NCORES = 8
DT = D // P   # 4 d-tiles
UT = U // P   # 4 u-tiles
ST = S // P   # 16 s-tiles
SC = S // 512  # 4 s-chunks
C_SHIFT = 8.0  # exp(s - C): keeps P' well inside fp8e4m3 range

_cache = {}


def _build():
    import concourse.mybir as mybir
    import concourse.tile as tile
    from concourse import bacc

    f32 = mybir.dt.float32
    bf16 = mybir.dt.bfloat16
    f8 = mybir.dt.float8e4
    AF = mybir.ActivationFunctionType
    DR = mybir.MatmulPerfMode.DoubleRow

    nc = bacc.Bacc("TRN2", target_bir_lowering=False, debug=False,
                   num_devices=NCORES)

    xt8_ext = nc.dram_tensor("xt8", [P, DT, S], f8, kind="ExternalInput")
    wq8_ext = nc.dram_tensor("wq8", [P, DT, U], f8, kind="ExternalInput")
    wv8_ext = nc.dram_tensor("wv8", [P, DT, U], f8, kind="ExternalInput")
    wk8_ext = nc.dram_tensor("wk8", [P, DT, U], f8, kind="ExternalInput")
    # out stored partition-major [p, s_tile, u] so each partition's DMA
    # run is 16 KiB contiguous (1 KiB rows would throttle the queues to
    # ~21 GB/s); the host unpermutes.
    out_ext = nc.dram_tensor("out", [P, ST, U], bf16, kind="ExternalOutput")
    den_ext = nc.dram_tensor("den", [1, S], f32, kind="ExternalOutput")

    # [k_p, q_f] additive mask for the diagonal block: keep (0) where
    # k < q strictly, -1e30 elsewhere.
    mask_dram = nc.inline_tensor(
        np.where(np.triu(np.ones((P, P), bool), 1), 0.0, -1e30)
        .astype(np.float32), "maskT_const")

    inv_sqrt_d = 1.0 / float(np.sqrt(D))

    with tile.TileContext(nc) as tc:
        with (
            tc.tile_pool(name="const", bufs=1) as constp,
            tc.tile_pool(name="inp", bufs=1) as inp,
            tc.tile_pool(name="proj", bufs=1) as projp,
            tc.tile_pool(name="pt", bufs=1) as ptp,
            tc.tile_pool(name="dhold", bufs=2) as dholdp,
        ):
            maskT = constp.tile([P, P], f32)
            nc.gpsimd.dma_start(maskT[:], mask_dram[:])
            # dual-fp8 LDWEIGHTS requires the standard paired stationary
            # shape; a [P, 2, 1] ones AP fails the ISA check, so keep a
            # [P, 2, 512] ones tile and use an M=128 slice (the denom
            # matmul then just produces 128 identical rows).
            ones8 = constp.tile([P, 2, 512], f8)
            nc.vector.memset(ones8[:], 1.0)
            nbias = constp.tile([P, 1], f32)
            nc.vector.memset(nbias[:], -C_SHIFT)

            # ---- input DMAs: per-queue bandwidth is only ~20-36 GB/s,
            # so spread everything partition-split across four engine
            # queues.  wq8 leads (first projection group); X^T next;
            # wk8/wv8 later (V and K^T phases come after Q^T). ----
            w8 = {n: inp.tile([P, DT, U], f8, name=f"w8_{n}")
                  for n in ("q", "v", "k")}
            xt8 = inp.tile([P, DT, S], f8, name="xt8")

            for lo, hi, qeng in ((0, 64, nc.sync), (64, P, nc.scalar)):
                qeng.dma_start(w8["q"][lo:hi], wq8_ext[lo:hi])
                qeng.dma_start(xt8[lo:hi, 0:2, :], xt8_ext[lo:hi, 0:2, :])
                qeng.dma_start(xt8[lo:hi, 2:4, :], xt8_ext[lo:hi, 2:4, :])
                qeng.dma_start(w8["v"][lo:hi], wv8_ext[lo:hi])
            nc.gpsimd.dma_start(w8["k"][:], wk8_ext[:])

            qT8 = projp.tile([P, UT, S], f8, name="qT8")
            kT8 = projp.tile([P, UT, S], f8, name="kT8")
            v8 = projp.tile([P, ST, U], f8, name="v8")
            outS = [projp.tile([P, 4, U], bf16, name=f"outS{qc}")
                    for qc in range(SC)]

            # P'^T pair tiles: pair jj covers key blocks (2jj, 2jj+1),
            # per 512-query chunk qc >= jj//2
            pt8 = {}
            for qc in range(SC):
                for jj in range(2 * qc + 2):
                    pt8[(jj, qc)] = ptp.tile([P, 2, 512], f8,
                                             name=f"pt8_{jj}_{qc}")

            denomT = dholdp.tile([1, S], f32, name="denomT")

            # ---- phase 1: projections (4-bank PSUM tiles, one sigmoid
            # per 2048 columns) ----
            with tc.tile_pool(name="bigps", bufs=2, space="PSUM") as bigps:
                def emit_qk(dst, wkey):
                    for u in range(UT):
                        ps = bigps.tile([P, SC, 512], f32, tag="big",
                                        name=f"ps_{u}_{wkey}")
                        for tt in (0, 2):
                            for c in range(SC):
                                nc.tensor.matmul(
                                    ps[:, c, :],
                                    w8[wkey][:, tt:tt + 2,
                                             u * P:(u + 1) * P],
                                    xt8[:, tt:tt + 2,
                                        c * 512:(c + 1) * 512],
                                    start=(tt == 0), stop=(tt == 2),
                                    perf_mode=DR)
                        nc.scalar.activation(out=dst[:, u, :], in_=ps[:],
                                             func=AF.Sigmoid)

                emit_qk(qT8, "q")
                for sg in range(4):  # V, groups of 4 s-tiles
                    ps = bigps.tile([P, 4, 512], f32, tag="big",
                                    name=f"ps_v_{sg}")
                    for si in range(4):
                        st = 4 * sg + si
                        for tt in (0, 2):
                            nc.tensor.matmul(
                                ps[:, si, :],
                                xt8[:, tt:tt + 2, st * P:(st + 1) * P],
                                w8["k"][:, tt:tt + 2, :],
                                start=(tt == 0), stop=(tt == 2),
                                perf_mode=DR)
                    nc.scalar.activation(out=v8[:, 4 * sg:4 * sg + 4, :],
                                         in_=ps[:], func=AF.Sigmoid)
                emit_qk(kT8, "v")

            # ---- phase 2: attention ----
            with (
                tc.tile_pool(name="sps", bufs=3, space="PSUM") as sps,
                tc.tile_pool(name="pvps", bufs=2, space="PSUM") as pvps,
            ):
                # scores S^T[k,q] grouped by key block j (stationary K^T
                # pair reused across q-chunks); after key-group g
                # finishes, chunk qc=g is fully scored -> denom + PV.
                def emit_scores(g):
                    for jj in (2 * g, 2 * g + 1):
                        ps = sps.tile([P, 2, 512], f32, tag="sps",
                                      name=f"ps_s_{jj}_{g}")
                        for hf in range(2):
                            j = 2 * jj + hf
                            r = j - 4 * g
                            # diagonal chunk (compact at col 0)
                            w = 512 - P * r
                            for uu in (0, 2):
                                nc.tensor.matmul(
                                    ps[:, hf, :w],
                                    kT8[:, uu:uu + 2, j * P:(j + 1) * P],
                                    qT8[:, uu:uu + 2,
                                        g * 512 + P * r:(g + 1) * 512],
                                    start=(uu == 0), stop=(uu == 2),
                                    perf_mode=DR)
                            nc.vector.tensor_add(
                                out=ps[:, hf, 0:P], in0=ps[:, hf, 0:P],
                                in1=maskT[:])
                            nc.scalar.activation(
                                out=pt8[(jj, g)][:, hf, P * r:512],
                                in_=ps[:, hf, :w], func=AF.Exp,
                                scale=inv_sqrt_d, bias=nbias[:, 0:1])
                            if r:
                                nc.vector.memset(
                                    pt8[(jj, g)][:, hf, 0:P * r], 0.0)
                        # later chunks: full width, exp two blocks at once
                        for qc in range(g + 1, SC):
                            ps = sps.tile([P, 2, 512], f32, tag="sps",
                                          name=f"ps_s_{jj}_{qc}")
                            for hf in range(2):
                                j = 2 * jj + hf
                                for uu in (0, 2):
                                    nc.tensor.matmul(
                                        ps[:, hf, :],
                                        kT8[:, uu:uu + 2,
                                            j * P:(j + 1) * P],
                                        qT8[:, uu:uu + 2,
                                            qc * 512:(qc + 1) * 512],
                                        start=(uu == 0), stop=(uu == 2),
                                        perf_mode=DR)
                            nc.scalar.activation(
                                out=pt8[(jj, qc)][:], in_=ps[:],
                                func=AF.Exp, scale=inv_sqrt_d,
                                bias=nbias[:, 0:1])

                def emit_chunk(qc):
                    npairs = 2 * qc + 2
                    # softmax denominators for this chunk
                    dn = pvps.tile([P, 512], f32, tag="pv",
                                   name=f"dn_{qc}")
                    for jj in range(npairs):
                        nc.tensor.matmul(
                            dn[:], ones8[:, :, 0:P], pt8[(jj, qc)][:],
                            start=(jj == 0), stop=(jj == npairs - 1),
                            perf_mode=DR)
                    nc.vector.tensor_copy(
                        out=denomT[:, qc * 512:(qc + 1) * 512],
                        in_=dn[0:1, :])
                    nc.gpsimd.dma_start(
                        den_ext[:, qc * 512:(qc + 1) * 512],
                        denomT[:, qc * 512:(qc + 1) * 512])
                    # PV: P'^T slice stationary -> natural [q, u]
                    for it in range(4):
                        i = 4 * qc + it
                        np_i = (i + 2) // 2
                        po = pvps.tile([P, U], f32, tag="pv")
                        for jj in range(np_i):
                            nc.tensor.matmul(
                                po[:],
                                pt8[(jj, qc)][:, :, it * P:(it + 1) * P],
                                v8[:, 2 * jj:2 * jj + 2, :],
                                start=(jj == 0), stop=(jj == np_i - 1),
                                perf_mode=DR)
                        nc.vector.tensor_copy(out=outS[qc][:, it, :],
                                              in_=po[:])
                        if qc == SC - 1:
                            # final chunk: stream per tile so the tail
                            # DMA is only the last 128 KiB
                            for lo, hi, qeng in ((0, 48, nc.sync),
                                                 (48, 96, nc.scalar),
                                                 (96, P, nc.gpsimd)):
                                qeng.dma_start(
                                    out_ext[lo:hi, i:i + 1, :],
                                    outS[qc][lo:hi, it:it + 1, :])
                    if qc < SC - 1:
                        for lo, hi, qeng in ((0, 48, nc.sync),
                                             (48, 96, nc.scalar),
                                             (96, P, nc.gpsimd)):
                            qeng.dma_start(
                                out_ext[lo:hi, 4 * qc:4 * qc + 4, :],
                                outS[qc][lo:hi, :, :])

                for g in range(SC):
                    emit_scores(g)
                    emit_chunk(g)

    nc.compile()
    return nc


def _get_nc():
    if "nc" not in _cache:
        _cache["nc"] = _build()
    return _cache["nc"]


def _prep_in_maps(query, Wq, Wv, Wk):
    import ml_dtypes

    f8 = ml_dtypes.float8_e4m3
    # X^T packed [128, 4, 2048]: xt8[p, t, s] = X[s, 128t + p]
    xt = np.ascontiguousarray(
        np.asarray(query, dtype=np.float32).transpose(0, 2, 1)
        .reshape(B, DT, P, S).transpose(0, 2, 1, 3)).astype(f8)
    ws = []
    for w in (Wq, Wv, Wk):
        ws.append(np.ascontiguousarray(
            np.asarray(w, dtype=np.float32)
            .reshape(DT, P, U).transpose(1, 0, 2)).astype(f8))
    wq8, wv8, wk8 = ws
    return [
        {"xt8": xt[b], "wq8": wq8, "wv8": wv8, "wk8": wk8}
        for b in range(B)
    ]


def _postprocess(res):
    out = np.empty((B, S, U), dtype=np.float32)
    for b in range(B):
        o = (np.asarray(res.results[b]["out"]).astype(np.float32)
             .transpose(1, 0, 2).reshape(S, U))
        den = np.asarray(res.results[b]["den"]).astype(np.float32)
        out[b] = o / np.maximum(den.reshape(S, 1), 1e-30)
    return out


def kernel(query, Wq, Wv, Wk):
    from concourse.bass_utils import run_bass_kernel_spmd

    nc = _get_nc()
    in_maps = _prep_in_maps(query, Wq, Wv, Wk)
    res = run_bass_kernel_spmd(nc, in_maps, core_ids=list(range(NCORES)))
    return _postprocess(res)
